# revision 11
# baseline (speedup 1.0000x reference)
"""Trainium2 Bass kernel for nn_MoEWrapper (8-expert top-2 MoE with unmerge proj).

Strategy (expert-parallel, sparse):
  Launch A: router logits on device, token-sharded over the 8 cores
            (fp32 matmul; top-2 selection gaps are ~5e-5 relative so the gate
            needs full fp32).
  Host:     softmax + top-2 + normalized combine weights; build per-expert token
            lists and gather/pad the (pre-transposed) activations — the
            all-to-all dispatch of the expert-parallel sharding, done as part
            of input sharding. Also folds unmerge into w2: w2u = w2[e] @ unm_e
            (pure weight preprocessing, done once per call).
  Launch B: core e runs expert e's FFN over its gathered tokens:
            hT = silu(w1.T x) * (w3.T x);  yT = (w2u.T @ hT) * w_combine
            All matmuls in float32r (full PE rate at >=256 moving rows,
            ~1e-4 rel err). Zero transposes on device: activations flow
            feature-major; the host untransposes the output during unshard.
  Host:     scatter-add the two expert contributions per token (unshard).
"""

import numpy as np

import concourse.mybir as mybir
import concourse.tile as tile
from concourse import bacc
from concourse.bass_utils import run_bass_kernel_spmd

P = 128
D = 1024          # hidden
F = 4096          # ffn
E = 8             # experts
NG = 2            # unmerge groups
GS = E // NG
NCORES = 8
CORE_IDS = list(range(NCORES))
N_TOK = 2 * 4096  # total tokens
NA = N_TOK // NCORES  # tokens per core in gate launch
KT = D // P       # 8  k-slices over hidden
FT = F // P       # 32 f-slices over ffn
DT8 = D // P      # 8  d_out slices
TOK_TILE = 768    # tokens per SBUF-resident tile in launch B

F32 = mybir.dt.float32

DT_MODE = "f32r"  # "f32r" | "bf16"

_PROGS = {}
LAST_PERF = {}


def _mm_dt(mode):
    return mybir.dt.float32r if mode == "f32r" else mybir.dt.bfloat16


def _np_dt(mode):
    if mode == "f32r":
        return np.float32
    import ml_dtypes
    return ml_dtypes.bfloat16


def _build_gate():
    """Per core: logitsT[E, NA] = gate_w.T @ xT_c  (fp32)."""
    nc = bacc.Bacc(None, target_bir_lowering=False)
    xt = nc.dram_tensor("xt", [D, NA], F32, kind="ExternalInput")
    gw = nc.dram_tensor("gw", [D, E], F32, kind="ExternalInput")
    lo = nc.dram_tensor("lo", [E, NA], F32, kind="ExternalOutput")

    xt3 = xt.rearrange("(kt p) n -> p kt n", p=P)
    gw3 = gw.rearrange("(kt p) e -> p kt e", p=P)

    with tile.TileContext(nc) as tc:
        with (
            tc.tile_pool(name="sb", bufs=1) as sb,
            tc.tile_pool(name="xb", bufs=2) as xb,
            tc.tile_pool(name="ps", bufs=2, space="PSUM") as ps,
            tc.tile_pool(name="ob", bufs=2) as ob,
        ):
            tg = sb.tile([P, KT, E], F32, tag="g")
            nc.sync.dma_start(tg[:], gw3[:])
            for ch in range(NA // 512):
                tx = xb.tile([P, KT, 512], F32, tag="x")
                for k in range(KT):
                    eng = (nc.sync, nc.gpsimd, nc.scalar)[k % 3]
                    eng.dma_start(tx[:, k, :], xt3[:, k, ch * 512:(ch + 1) * 512])
                pp = ps.tile([E, 512], F32, tag="p")
                for k in range(KT):
                    nc.tensor.matmul(
                        pp[:], tg[:, k, :], tx[:, k, :],
                        start=(k == 0), stop=(k == KT - 1),
                    )
                to = ob.tile([E, 512], F32, tag="o")
                nc.any.tensor_copy(out=to[:], in_=pp[:])
                nc.sync.dma_start(lo[:, ch * 512:(ch + 1) * 512], to[:])
    nc.compile()
    return nc


def _token_tiles(C):
    """Split capacity C (multiple of 128) into tiles of <= TOK_TILE."""
    tiles = []
    t0 = 0
    while t0 < C:
        tn = min(TOK_TILE, C - t0)
        tiles.append((t0, tn))
        t0 += tn
    # put the (possibly short) remainder tile first: its smaller matmuls land
    # in the HAM-cold window where they cost the same as full ones
    tiles.sort(key=lambda t: t[1])
    return tiles


def _chunks(tn):
    """Split a token tile into PSUM-sized chunks (<=512 each)."""
    out = []
    c0 = 0
    while c0 < tn:
        cn = min(512, tn - c0)
        out.append((c0, cn))
        c0 += cn
    return out


def _build_expert(C, mode):
    """Per core: yT[D, C] = (w2u.T @ (silu(w1.T x) * (w3.T x))) * wc."""
    dt = _mm_dt(mode)
    nc = bacc.Bacc(None, target_bir_lowering=False)
    xt = nc.dram_tensor("xt", [D, C], dt, kind="ExternalInput")
    w1s = nc.dram_tensor("w1s", [FT, D, P], dt, kind="ExternalInput")
    w3s = nc.dram_tensor("w3s", [FT, D, P], dt, kind="ExternalInput")
    w2s = nc.dram_tensor("w2s", [DT8, F, P], dt, kind="ExternalInput")  # folded w2@unm
    wcb = nc.dram_tensor("wcb", [P, C], F32, kind="ExternalInput")      # combine w, bcast
    yt = nc.dram_tensor("yt", [D, C], F32, kind="ExternalOutput")

    xt3 = xt.rearrange("(kt p) n -> p kt n", p=P)            # [P, KT, C]
    w1r = w1s.rearrange("ft (kt p) m -> ft p kt m", p=P)     # [FT, P, KT, P]
    w3r = w3s.rearrange("ft (kt p) m -> ft p kt m", p=P)
    w2r = w2s.rearrange("dt (ft p) m -> dt p ft m", p=P)     # [DT8, P, FT, P]
    yt3 = yt.rearrange("(dt p) n -> p dt n", p=P)            # [P, DT8, C]

    silu = mybir.ActivationFunctionType.Silu

    with tile.TileContext(nc) as tc:
        with (
            tc.tile_pool(name="xp", bufs=1) as xp,
            tc.tile_pool(name="wp", bufs=4) as wp,
            tc.tile_pool(name="hp", bufs=1) as hp,
            tc.tile_pool(name="cp", bufs=2) as cp,
            tc.tile_pool(name="sp", bufs=4) as sp,
            tc.tile_pool(name="zp", bufs=4) as zp,
            tc.tile_pool(name="ps", bufs=2, space="PSUM") as ps,
        ):
            for (t0, tn) in _token_tiles(C):
                chunks = _chunks(tn)
                tx = xp.tile([P, KT, TOK_TILE], dt, tag="x")
                for k in range(KT):
                    eng = (nc.gpsimd, nc.scalar, nc.sync)[k % 3]
                    eng.dma_start(tx[:, k, :tn], xt3[:, k, t0:t0 + tn])
                twcb = cp.tile([P, TOK_TILE], F32, tag="wc")
                nc.gpsimd.dma_start(twcb[:, :tn], wcb[:, t0:t0 + tn])

                # Stage 1: hT[f, tok] = silu(w1.T x) * (w3.T x)
                th = hp.tile([P, FT, TOK_TILE], dt, tag="h")
                for ft in range(FT):
                    tw1 = wp.tile([P, KT, P], dt, tag="w1")
                    nc.sync.dma_start(tw1[:], w1r[ft])
                    tw3 = wp.tile([P, KT, P], dt, tag="w3")
                    nc.sync.dma_start(tw3[:], w3r[ft])
                    pa = [ps.tile([P, 512], F32, tag="a", name=f"pa{ci}") for ci in range(len(chunks))]
                    pb = [ps.tile([P, 512], F32, tag="b", name=f"pb{ci}") for ci in range(len(chunks))]
                    for k in range(KT):
                        st, sp_ = (k == 0), (k == KT - 1)
                        for ci, (c0, cn) in enumerate(chunks):
                            nc.tensor.matmul(
                                pa[ci][:, :cn], tw1[:, k, :],
                                tx[:, k, c0:c0 + cn], start=st, stop=sp_)
                        for ci, (c0, cn) in enumerate(chunks):
                            nc.tensor.matmul(
                                pb[ci][:, :cn], tw3[:, k, :],
                                tx[:, k, c0:c0 + cn], start=st, stop=sp_)
                    for ci, (c0, cn) in enumerate(chunks):
                        tsl = sp.tile([P, 512], F32, tag="s")
                        nc.scalar.activation(tsl[:, :cn], pa[ci][:, :cn], silu)
                        nc.vector.tensor_mul(out=th[:, ft, c0:c0 + cn],
                                             in0=tsl[:, :cn], in1=pb[ci][:, :cn])

                # Stage 2: yT[d, tok] = (w2u.T @ hT) * wc
                tw2_pre = []
                for q in range(4):
                    tw2 = wp.tile([P, 8, P], dt, tag="w2", name=f"tw2p{q}")
                    nc.sync.dma_start(tw2[:], w2r[0][:, q * 8:(q + 1) * 8, :])
                    tw2_pre.append(tw2)
                for d8 in range(DT8):
                    py = [ps.tile([P, 512], F32, tag="y", name=f"py{ci}", bufs=4) for ci in range(len(chunks))]
                    for q in range(4):
                        if d8 == 0:
                            tw2 = tw2_pre[q]
                        else:
                            tw2 = wp.tile([P, 8, P], dt, tag="w2")
                            nc.sync.dma_start(tw2[:], w2r[d8][:, q * 8:(q + 1) * 8, :])
                        for k8 in range(8):
                            st, sp_ = (q == 0 and k8 == 0), (q == 3 and k8 == 7)
                            for ci, (c0, cn) in enumerate(chunks):
                                nc.tensor.matmul(
                                    py[ci][:, :cn], tw2[:, k8, :],
                                    th[:, q * 8 + k8, c0:c0 + cn],
                                    start=st, stop=sp_)
                    for ci, (c0, cn) in enumerate(chunks):
                        tz = zp.tile([P, 512], F32, tag="zo")
                        nc.vector.tensor_mul(out=tz[:, :cn], in0=py[ci][:, :cn],
                                             in1=twcb[:, c0:c0 + cn])
                        nc.sync.dma_start(yt3[:, d8, t0 + c0:t0 + c0 + cn],
                                          tz[:, :cn])
    nc.compile()
    return nc


def _get_prog(key, builder):
    if key not in _PROGS:
        _PROGS[key] = builder()
    return _PROGS[key]


def kernel(**inputs):
    hs = np.ascontiguousarray(np.asarray(inputs["hidden_states"], dtype=np.float32))
    gw = np.ascontiguousarray(np.asarray(inputs["gate_w"], dtype=np.float32))
    w1 = np.asarray(inputs["w1"], dtype=np.float32)
    w3 = np.asarray(inputs["w3"], dtype=np.float32)
    w2 = np.asarray(inputs["w2"], dtype=np.float32)
    unmerge = np.asarray(inputs["unmerge"], dtype=np.float32)

    b, s, d = hs.shape
    n = b * s
    x = hs.reshape(n, d)
    xT = np.ascontiguousarray(x.T)  # [D, n]

    # ---- Launch A: router logits ----
    ncA = _get_prog("gate", _build_gate)
    in_maps = [
        {"xt": np.ascontiguousarray(xT[:, c * NA:(c + 1) * NA]), "gw": gw}
        for c in range(NCORES)
    ]
    resA = run_bass_kernel_spmd(ncA, in_maps, CORE_IDS)
    LAST_PERF["gate"] = resA
    logits = np.concatenate([resA.results[c]["lo"] for c in range(NCORES)], axis=1)
    router_logits = np.ascontiguousarray(logits.T)  # [n, E]

    # ---- Host: top-2 routing / dispatch (the expert-parallel all-to-all) ----
    m = router_logits.max(axis=-1, keepdims=True)
    ex = np.exp(router_logits - m)
    rw = ex / ex.sum(axis=-1, keepdims=True)
    sel = np.argpartition(-rw, 1, axis=-1)[:, :2]           # top-2 experts per token
    tw = np.take_along_axis(rw, sel, axis=-1)
    tw = tw / tw.sum(axis=-1, keepdims=True)

    idx_e, wt_e = [], []
    for e in range(E):
        hit = (sel == e)
        rows = np.nonzero(hit.any(axis=-1))[0]
        wsel = np.where(hit[rows, 0], tw[rows, 0], tw[rows, 1])
        idx_e.append(rows)
        wt_e.append(wsel.astype(np.float32))
    counts = np.array([len(r) for r in idx_e])
    C = int(-(-counts.max() // P) * P)  # round up to multiple of 128

    # ---- Launch B: expert FFN chains ----
    mode = DT_MODE
    npdt = _np_dt(mode)
    ncB = _get_prog(("expert", C, mode), lambda: _build_expert(C, mode))
    in_maps = []
    for e in range(E):
        g, gi = e // GS, e % GS
        unm_e = unmerge[g][:, gi * D:(gi + 1) * D]
        w2u = w2[e] @ unm_e                                  # fold unmerge into w2
        xg = np.zeros((D, C), dtype=np.float32)
        xg[:, :counts[e]] = xT[:, idx_e[e]]
        wc = np.zeros((C,), dtype=np.float32)
        wc[:counts[e]] = wt_e[e]
        wcb = np.ascontiguousarray(np.broadcast_to(wc[None, :], (P, C)))
        w1s = np.ascontiguousarray(
            w1[e].reshape(D, FT, P).transpose(1, 0, 2)).astype(npdt, copy=False)
        w3s = np.ascontiguousarray(
            w3[e].reshape(D, FT, P).transpose(1, 0, 2)).astype(npdt, copy=False)
        w2s = np.ascontiguousarray(
            w2u.reshape(F, DT8, P).transpose(1, 0, 2)).astype(npdt, copy=False)
        in_maps.append({
            "xt": xg.astype(npdt, copy=False),
            "w1s": w1s, "w3s": w3s, "w2s": w2s, "wcb": wcb,
        })
    resB = run_bass_kernel_spmd(ncB, in_maps, CORE_IDS)
    LAST_PERF["expert"] = resB

    # ---- Host: unshard (scatter-add the two expert contributions per token) ----
    outT = np.zeros((d, n), dtype=np.float32)
    for e in range(E):
        outT[:, idx_e[e]] += resB.results[e]["yt"][:, :counts[e]]
    out = np.ascontiguousarray(outT.T)

    return out.reshape(b, s, d), router_logits


# revision 12
# speedup vs baseline: 1.0011x; 1.0011x over previous
"""Trainium2 Bass kernel for nn_MoEWrapper (8-expert top-2 MoE with unmerge proj).

Strategy (expert-parallel, sparse):
  Launch A: router logits on device, token-sharded over the 8 cores
            (fp32 matmul; top-2 selection gaps are ~5e-5 relative so the gate
            needs full fp32).
  Host:     softmax + top-2 + normalized combine weights; build per-expert token
            lists and gather/pad the (pre-transposed) activations — the
            all-to-all dispatch of the expert-parallel sharding, done as part
            of input sharding. Also folds unmerge into w2: w2u = w2[e] @ unm_e
            (pure weight preprocessing, done once per call).
  Launch B: core e runs expert e's FFN over its gathered tokens:
            hT = silu(w1.T x) * (w3.T x);  yT = (w2u.T @ hT) * w_combine
            All matmuls in float32r (full PE rate at >=256 moving rows,
            ~1e-4 rel err). Zero transposes on device: activations flow
            feature-major; the host untransposes the output during unshard.
  Host:     scatter-add the two expert contributions per token (unshard).
"""

import numpy as np

import concourse.mybir as mybir
import concourse.tile as tile
from concourse import bacc
from concourse.bass_utils import run_bass_kernel_spmd

P = 128
D = 1024          # hidden
F = 4096          # ffn
E = 8             # experts
NG = 2            # unmerge groups
GS = E // NG
NCORES = 8
CORE_IDS = list(range(NCORES))
N_TOK = 2 * 4096  # total tokens
NA = N_TOK // NCORES  # tokens per core in gate launch
KT = D // P       # 8  k-slices over hidden
FT = F // P       # 32 f-slices over ffn
DT8 = D // P      # 8  d_out slices
TOK_TILE = 768    # tokens per SBUF-resident tile in launch B

F32 = mybir.dt.float32

DT_MODE = "f32r"  # "f32r" | "bf16"

_PROGS = {}
LAST_PERF = {}


def _mm_dt(mode):
    return mybir.dt.float32r if mode == "f32r" else mybir.dt.bfloat16


def _np_dt(mode):
    if mode == "f32r":
        return np.float32
    import ml_dtypes
    return ml_dtypes.bfloat16


def _build_gate():
    """Per core: logitsT[E, NA] = gate_w.T @ xT_c  (fp32)."""
    nc = bacc.Bacc(None, target_bir_lowering=False)
    xt = nc.dram_tensor("xt", [D, NA], F32, kind="ExternalInput")
    gw = nc.dram_tensor("gw", [D, E], F32, kind="ExternalInput")
    lo = nc.dram_tensor("lo", [E, NA], F32, kind="ExternalOutput")

    xt3 = xt.rearrange("(kt p) n -> p kt n", p=P)
    gw3 = gw.rearrange("(kt p) e -> p kt e", p=P)

    with tile.TileContext(nc) as tc:
        with (
            tc.tile_pool(name="sb", bufs=1) as sb,
            tc.tile_pool(name="xb", bufs=2) as xb,
            tc.tile_pool(name="ps", bufs=2, space="PSUM") as ps,
            tc.tile_pool(name="ob", bufs=2) as ob,
        ):
            tg = sb.tile([P, KT, E], F32, tag="g")
            nc.sync.dma_start(tg[:], gw3[:])
            for ch in range(NA // 512):
                tx = xb.tile([P, KT, 512], F32, tag="x")
                for k in range(KT):
                    eng = (nc.sync, nc.gpsimd, nc.scalar)[k % 3]
                    eng.dma_start(tx[:, k, :], xt3[:, k, ch * 512:(ch + 1) * 512])
                pp = ps.tile([E, 512], F32, tag="p")
                for k in range(KT):
                    nc.tensor.matmul(
                        pp[:], tg[:, k, :], tx[:, k, :],
                        start=(k == 0), stop=(k == KT - 1),
                    )
                to = ob.tile([E, 512], F32, tag="o")
                nc.any.tensor_copy(out=to[:], in_=pp[:])
                nc.sync.dma_start(lo[:, ch * 512:(ch + 1) * 512], to[:])
    nc.compile()
    return nc


def _token_tiles(C):
    """Split capacity C (multiple of 128) into tiles of <= TOK_TILE."""
    tiles = []
    t0 = 0
    while t0 < C:
        tn = min(TOK_TILE, C - t0)
        tiles.append((t0, tn))
        t0 += tn
    # put the (possibly short) remainder tile first: its smaller matmuls land
    # in the HAM-cold window where they cost the same as full ones
    tiles.sort(key=lambda t: t[1])
    return tiles


def _chunks(tn):
    """Split a token tile into PSUM-sized chunks (<=512 each)."""
    out = []
    c0 = 0
    while c0 < tn:
        cn = min(512, tn - c0)
        out.append((c0, cn))
        c0 += cn
    return out


def _build_expert(C, mode):
    """Per core: yT[D, C] = (w2u.T @ (silu(w1.T x) * (w3.T x))) * wc."""
    dt = _mm_dt(mode)
    nc = bacc.Bacc(None, target_bir_lowering=False)
    xt = nc.dram_tensor("xt", [D, C], dt, kind="ExternalInput")
    w1s = nc.dram_tensor("w1s", [FT, P, KT, P], dt, kind="ExternalInput")
    w3s = nc.dram_tensor("w3s", [FT, P, KT, P], dt, kind="ExternalInput")
    w2s = nc.dram_tensor("w2s", [DT8, P, FT, P], dt, kind="ExternalInput")  # folded w2@unm
    wcb = nc.dram_tensor("wcb", [P, C], F32, kind="ExternalInput")      # combine w, bcast
    yt = nc.dram_tensor("yt", [D, C], F32, kind="ExternalOutput")

    xt3 = xt.rearrange("(kt p) n -> p kt n", p=P)            # [P, KT, C]
    w1r, w3r, w2r = w1s, w3s, w2s                            # already SBUF layout
    yt3 = yt.rearrange("(dt p) n -> p dt n", p=P)            # [P, DT8, C]

    silu = mybir.ActivationFunctionType.Silu

    with tile.TileContext(nc) as tc:
        with (
            tc.tile_pool(name="xp", bufs=1) as xp,
            tc.tile_pool(name="wp", bufs=4) as wp,
            tc.tile_pool(name="hp", bufs=1) as hp,
            tc.tile_pool(name="cp", bufs=2) as cp,
            tc.tile_pool(name="sp", bufs=4) as sp,
            tc.tile_pool(name="zp", bufs=4) as zp,
            tc.tile_pool(name="ps", bufs=2, space="PSUM") as ps,
        ):
            for (t0, tn) in _token_tiles(C):
                chunks = _chunks(tn)
                tx = xp.tile([P, KT, TOK_TILE], dt, tag="x")
                for k in range(KT):
                    eng = (nc.gpsimd, nc.scalar, nc.sync)[k % 3]
                    eng.dma_start(tx[:, k, :tn], xt3[:, k, t0:t0 + tn])
                twcb = cp.tile([P, TOK_TILE], F32, tag="wc")
                nc.gpsimd.dma_start(twcb[:, :tn], wcb[:, t0:t0 + tn])

                # Stage 1: hT[f, tok] = silu(w1.T x) * (w3.T x)
                th = hp.tile([P, FT, TOK_TILE], dt, tag="h")
                for ft in range(FT):
                    tw1 = wp.tile([P, KT, P], dt, tag="w1")
                    nc.sync.dma_start(tw1[:], w1r[ft])
                    tw3 = wp.tile([P, KT, P], dt, tag="w3")
                    nc.sync.dma_start(tw3[:], w3r[ft])
                    pa = [ps.tile([P, 512], F32, tag="a", name=f"pa{ci}") for ci in range(len(chunks))]
                    pb = [ps.tile([P, 512], F32, tag="b", name=f"pb{ci}") for ci in range(len(chunks))]
                    for k in range(KT):
                        st, sp_ = (k == 0), (k == KT - 1)
                        for ci, (c0, cn) in enumerate(chunks):
                            nc.tensor.matmul(
                                pa[ci][:, :cn], tw1[:, k, :],
                                tx[:, k, c0:c0 + cn], start=st, stop=sp_)
                        for ci, (c0, cn) in enumerate(chunks):
                            nc.tensor.matmul(
                                pb[ci][:, :cn], tw3[:, k, :],
                                tx[:, k, c0:c0 + cn], start=st, stop=sp_)
                    for ci, (c0, cn) in enumerate(chunks):
                        tsl = sp.tile([P, 512], F32, tag="s")
                        nc.scalar.activation(tsl[:, :cn], pa[ci][:, :cn], silu)
                        nc.vector.tensor_mul(out=th[:, ft, c0:c0 + cn],
                                             in0=tsl[:, :cn], in1=pb[ci][:, :cn])

                # Stage 2: yT[d, tok] = (w2u.T @ hT) * wc
                tw2_pre = []
                for q in range(4):
                    tw2 = wp.tile([P, 8, P], dt, tag="w2", name=f"tw2p{q}")
                    nc.sync.dma_start(tw2[:], w2r[0][:, q * 8:(q + 1) * 8, :])
                    tw2_pre.append(tw2)
                for d8 in range(DT8):
                    py = [ps.tile([P, 512], F32, tag="y", name=f"py{ci}", bufs=4) for ci in range(len(chunks))]
                    for q in range(4):
                        if d8 == 0:
                            tw2 = tw2_pre[q]
                        else:
                            tw2 = wp.tile([P, 8, P], dt, tag="w2")
                            nc.sync.dma_start(tw2[:], w2r[d8][:, q * 8:(q + 1) * 8, :])
                        for k8 in range(8):
                            st, sp_ = (q == 0 and k8 == 0), (q == 3 and k8 == 7)
                            for ci, (c0, cn) in enumerate(chunks):
                                nc.tensor.matmul(
                                    py[ci][:, :cn], tw2[:, k8, :],
                                    th[:, q * 8 + k8, c0:c0 + cn],
                                    start=st, stop=sp_)
                    for ci, (c0, cn) in enumerate(chunks):
                        tz = zp.tile([P, 512], F32, tag="zo")
                        nc.vector.tensor_mul(out=tz[:, :cn], in0=py[ci][:, :cn],
                                             in1=twcb[:, c0:c0 + cn])
                        nc.sync.dma_start(yt3[:, d8, t0 + c0:t0 + c0 + cn],
                                          tz[:, :cn])
    nc.compile()
    return nc


def _get_prog(key, builder):
    if key not in _PROGS:
        _PROGS[key] = builder()
    return _PROGS[key]


def kernel(**inputs):
    hs = np.ascontiguousarray(np.asarray(inputs["hidden_states"], dtype=np.float32))
    gw = np.ascontiguousarray(np.asarray(inputs["gate_w"], dtype=np.float32))
    w1 = np.asarray(inputs["w1"], dtype=np.float32)
    w3 = np.asarray(inputs["w3"], dtype=np.float32)
    w2 = np.asarray(inputs["w2"], dtype=np.float32)
    unmerge = np.asarray(inputs["unmerge"], dtype=np.float32)

    b, s, d = hs.shape
    n = b * s
    x = hs.reshape(n, d)
    xT = np.ascontiguousarray(x.T)  # [D, n]

    # ---- Launch A: router logits ----
    ncA = _get_prog("gate", _build_gate)
    in_maps = [
        {"xt": np.ascontiguousarray(xT[:, c * NA:(c + 1) * NA]), "gw": gw}
        for c in range(NCORES)
    ]
    resA = run_bass_kernel_spmd(ncA, in_maps, CORE_IDS)
    LAST_PERF["gate"] = resA
    logits = np.concatenate([resA.results[c]["lo"] for c in range(NCORES)], axis=1)
    router_logits = np.ascontiguousarray(logits.T)  # [n, E]

    # ---- Host: top-2 routing / dispatch (the expert-parallel all-to-all) ----
    m = router_logits.max(axis=-1, keepdims=True)
    ex = np.exp(router_logits - m)
    rw = ex / ex.sum(axis=-1, keepdims=True)
    sel = np.argpartition(-rw, 1, axis=-1)[:, :2]           # top-2 experts per token
    tw = np.take_along_axis(rw, sel, axis=-1)
    tw = tw / tw.sum(axis=-1, keepdims=True)

    idx_e, wt_e = [], []
    for e in range(E):
        hit = (sel == e)
        rows = np.nonzero(hit.any(axis=-1))[0]
        wsel = np.where(hit[rows, 0], tw[rows, 0], tw[rows, 1])
        idx_e.append(rows)
        wt_e.append(wsel.astype(np.float32))
    counts = np.array([len(r) for r in idx_e])
    C = int(-(-counts.max() // P) * P)  # round up to multiple of 128

    # ---- Launch B: expert FFN chains ----
    mode = DT_MODE
    npdt = _np_dt(mode)
    ncB = _get_prog(("expert", C, mode), lambda: _build_expert(C, mode))
    in_maps = []
    for e in range(E):
        g, gi = e // GS, e % GS
        unm_e = unmerge[g][:, gi * D:(gi + 1) * D]
        w2u = w2[e] @ unm_e                                  # fold unmerge into w2
        xg = np.zeros((D, C), dtype=np.float32)
        xg[:, :counts[e]] = xT[:, idx_e[e]]
        wc = np.zeros((C,), dtype=np.float32)
        wc[:counts[e]] = wt_e[e]
        wcb = np.ascontiguousarray(np.broadcast_to(wc[None, :], (P, C)))
        w1s = np.ascontiguousarray(
            w1[e].reshape(KT, P, FT, P).transpose(2, 1, 0, 3)).astype(npdt, copy=False)
        w3s = np.ascontiguousarray(
            w3[e].reshape(KT, P, FT, P).transpose(2, 1, 0, 3)).astype(npdt, copy=False)
        w2s = np.ascontiguousarray(
            w2u.reshape(FT, P, DT8, P).transpose(2, 1, 0, 3)).astype(npdt, copy=False)
        in_maps.append({
            "xt": xg.astype(npdt, copy=False),
            "w1s": w1s, "w3s": w3s, "w2s": w2s, "wcb": wcb,
        })
    resB = run_bass_kernel_spmd(ncB, in_maps, CORE_IDS)
    LAST_PERF["expert"] = resB

    # ---- Host: unshard (scatter-add the two expert contributions per token) ----
    outT = np.zeros((d, n), dtype=np.float32)
    for e in range(E):
        outT[:, idx_e[e]] += resB.results[e]["yt"][:, :counts[e]]
    out = np.ascontiguousarray(outT.T)

    return out.reshape(b, s, d), router_logits


# revision 13
# speedup vs baseline: 1.0209x; 1.0198x over previous
"""Trainium2 Bass kernel for nn_MoEWrapper (8-expert top-2 MoE with unmerge proj).

Strategy (expert-parallel, sparse):
  Launch A: router logits on device, token-sharded over the 8 cores
            (fp32 matmul; top-2 selection gaps are ~5e-5 relative so the gate
            needs full fp32).
  Host:     softmax + top-2 + normalized combine weights; build per-expert token
            lists and gather/pad the (pre-transposed) activations — the
            all-to-all dispatch of the expert-parallel sharding, done as part
            of input sharding. Also folds unmerge into w2: w2u = w2[e] @ unm_e
            (pure weight preprocessing, done once per call).
  Launch B: core e runs expert e's FFN over its gathered tokens:
            hT = silu(w1.T x) * (w3.T x);  yT = (w2u.T @ hT) * w_combine
            All matmuls in float32r (full PE rate at >=256 moving rows,
            ~1e-4 rel err). Zero transposes on device: activations flow
            feature-major; the host untransposes the output during unshard.
  Host:     scatter-add the two expert contributions per token (unshard).
"""

import numpy as np

import concourse.mybir as mybir
import concourse.tile as tile
from concourse import bacc
from concourse.bass_utils import run_bass_kernel_spmd

P = 128
D = 1024          # hidden
F = 4096          # ffn
E = 8             # experts
NG = 2            # unmerge groups
GS = E // NG
NCORES = 8
CORE_IDS = list(range(NCORES))
N_TOK = 2 * 4096  # total tokens
NA = N_TOK // NCORES  # tokens per core in gate launch
KT = D // P       # 8  k-slices over hidden
FT = F // P       # 32 f-slices over ffn
DT8 = D // P      # 8  d_out slices
TOK_TILE = 768    # tokens per SBUF-resident tile in launch B

F32 = mybir.dt.float32

DT_MODE = "f32r"  # "f32r" | "bf16"

_PROGS = {}
LAST_PERF = {}


def _mm_dt(mode):
    return mybir.dt.float32r if mode == "f32r" else mybir.dt.bfloat16


def _np_dt(mode):
    if mode == "f32r":
        return np.float32
    import ml_dtypes
    return ml_dtypes.bfloat16


def _build_gate():
    """Per core: logitsT[E, NA] = gate_w.T @ xT_c  (fp32)."""
    nc = bacc.Bacc(None, target_bir_lowering=False)
    xt = nc.dram_tensor("xt", [D, NA], F32, kind="ExternalInput")
    gw = nc.dram_tensor("gw", [D, E], F32, kind="ExternalInput")
    lo = nc.dram_tensor("lo", [E, NA], F32, kind="ExternalOutput")

    xt3 = xt.rearrange("(kt p) n -> p kt n", p=P)
    gw3 = gw.rearrange("(kt p) e -> p kt e", p=P)

    with tile.TileContext(nc) as tc:
        with (
            tc.tile_pool(name="sb", bufs=1) as sb,
            tc.tile_pool(name="xb", bufs=2) as xb,
            tc.tile_pool(name="ps", bufs=2, space="PSUM") as ps,
            tc.tile_pool(name="ob", bufs=2) as ob,
        ):
            tg = sb.tile([P, KT, E], F32, tag="g")
            nc.sync.dma_start(tg[:], gw3[:])
            for ch in range(NA // 512):
                tx = xb.tile([P, KT, 512], F32, tag="x")
                for k in range(KT):
                    eng = (nc.sync, nc.gpsimd, nc.scalar)[k % 3]
                    eng.dma_start(tx[:, k, :], xt3[:, k, ch * 512:(ch + 1) * 512])
                pp = ps.tile([E, 512], F32, tag="p")
                for k in range(KT):
                    nc.tensor.matmul(
                        pp[:], tg[:, k, :], tx[:, k, :],
                        start=(k == 0), stop=(k == KT - 1),
                    )
                to = ob.tile([E, 512], F32, tag="o")
                nc.any.tensor_copy(out=to[:], in_=pp[:])
                nc.sync.dma_start(lo[:, ch * 512:(ch + 1) * 512], to[:])
    nc.compile()
    return nc


def _token_tiles(C):
    """Split capacity C (multiple of 128) into tiles of <= TOK_TILE."""
    tiles = []
    t0 = 0
    while t0 < C:
        tn = min(TOK_TILE, C - t0)
        tiles.append((t0, tn))
        t0 += tn
    # put the (possibly short) remainder tile first: its smaller matmuls land
    # in the HAM-cold window where they cost the same as full ones
    tiles.sort(key=lambda t: t[1])
    return tiles


def _chunks(tn):
    """Split a token tile into PSUM-sized chunks (<=512 each)."""
    out = []
    c0 = 0
    while c0 < tn:
        cn = min(512, tn - c0)
        out.append((c0, cn))
        c0 += cn
    return out


def _build_expert(C, mode):
    """Per core: yT[D, C] = (w2u.T @ (silu(w1.T x) * (w3.T x))) * wc."""
    dt = _mm_dt(mode)
    nc = bacc.Bacc(None, target_bir_lowering=False)
    xt = nc.dram_tensor("xt", [D, C], dt, kind="ExternalInput")
    w1s = nc.dram_tensor("w1s", [FT, P, KT, P], dt, kind="ExternalInput")
    w3s = nc.dram_tensor("w3s", [FT, P, KT, P], dt, kind="ExternalInput")
    w2s = nc.dram_tensor("w2s", [DT8, P, FT, P], dt, kind="ExternalInput")  # folded w2@unm
    wcb = nc.dram_tensor("wcb", [P, C], F32, kind="ExternalInput")      # combine w, bcast
    yt = nc.dram_tensor("yt", [D, C], F32, kind="ExternalOutput")

    xt3 = xt.rearrange("(kt p) n -> p kt n", p=P)            # [P, KT, C]
    w1r, w3r, w2r = w1s, w3s, w2s                            # already SBUF layout
    yt3 = yt.rearrange("(dt p) n -> p dt n", p=P)            # [P, DT8, C]

    silu = mybir.ActivationFunctionType.Silu

    with tile.TileContext(nc) as tc:
        with (
            tc.tile_pool(name="xp", bufs=1) as xp,
            tc.tile_pool(name="wp", bufs=4) as wp,
            tc.tile_pool(name="hp", bufs=1) as hp,
            tc.tile_pool(name="cp", bufs=2) as cp,
            tc.tile_pool(name="sp", bufs=4) as sp,
            tc.tile_pool(name="zp", bufs=4) as zp,
            tc.tile_pool(name="ps", bufs=2, space="PSUM") as ps,
        ):
            for (t0, tn) in _token_tiles(C):
                chunks = _chunks(tn)
                tx = xp.tile([P, KT, TOK_TILE], dt, tag="x")
                for k in range(KT):
                    nc.sync.dma_start(tx[:, k, :tn], xt3[:, k, t0:t0 + tn])
                twcb = cp.tile([P, TOK_TILE], F32, tag="wc")
                nc.sync.dma_start(twcb[:, :tn], wcb[:, t0:t0 + tn])

                # Stage 1: hT[f, tok] = silu(w1.T x) * (w3.T x)
                th = hp.tile([P, FT, TOK_TILE], dt, tag="h")
                for ft in range(FT):
                    tw1 = wp.tile([P, KT, P], dt, tag="w1")
                    nc.sync.dma_start(tw1[:], w1r[ft])
                    tw3 = wp.tile([P, KT, P], dt, tag="w3")
                    nc.sync.dma_start(tw3[:], w3r[ft])
                    pa = [ps.tile([P, 512], F32, tag="a", name=f"pa{ci}") for ci in range(len(chunks))]
                    pb = [ps.tile([P, 512], F32, tag="b", name=f"pb{ci}") for ci in range(len(chunks))]
                    for k in range(KT):
                        st, sp_ = (k == 0), (k == KT - 1)
                        for ci, (c0, cn) in enumerate(chunks):
                            nc.tensor.matmul(
                                pa[ci][:, :cn], tw1[:, k, :],
                                tx[:, k, c0:c0 + cn], start=st, stop=sp_)
                        for ci, (c0, cn) in enumerate(chunks):
                            nc.tensor.matmul(
                                pb[ci][:, :cn], tw3[:, k, :],
                                tx[:, k, c0:c0 + cn], start=st, stop=sp_)
                    for ci, (c0, cn) in enumerate(chunks):
                        tsl = sp.tile([P, 512], F32, tag="s")
                        nc.scalar.activation(tsl[:, :cn], pa[ci][:, :cn], silu)
                        nc.vector.tensor_mul(out=th[:, ft, c0:c0 + cn],
                                             in0=tsl[:, :cn], in1=pb[ci][:, :cn])

                # Stage 2: yT[d, tok] = (w2u.T @ hT) * wc
                tw2_pre = []
                for q in range(4):
                    tw2 = wp.tile([P, 8, P], dt, tag="w2", name=f"tw2p{q}")
                    nc.sync.dma_start(tw2[:], w2r[0][:, q * 8:(q + 1) * 8, :])
                    tw2_pre.append(tw2)
                for d8 in range(DT8):
                    py = [ps.tile([P, 512], F32, tag="y", name=f"py{ci}", bufs=4) for ci in range(len(chunks))]
                    for q in range(4):
                        if d8 == 0:
                            tw2 = tw2_pre[q]
                        else:
                            tw2 = wp.tile([P, 8, P], dt, tag="w2")
                            nc.sync.dma_start(tw2[:], w2r[d8][:, q * 8:(q + 1) * 8, :])
                        for k8 in range(8):
                            st, sp_ = (q == 0 and k8 == 0), (q == 3 and k8 == 7)
                            for ci, (c0, cn) in enumerate(chunks):
                                nc.tensor.matmul(
                                    py[ci][:, :cn], tw2[:, k8, :],
                                    th[:, q * 8 + k8, c0:c0 + cn],
                                    start=st, stop=sp_)
                    for ci, (c0, cn) in enumerate(chunks):
                        tz = zp.tile([P, 512], F32, tag="zo")
                        nc.vector.tensor_mul(out=tz[:, :cn], in0=py[ci][:, :cn],
                                             in1=twcb[:, c0:c0 + cn])
                        nc.sync.dma_start(yt3[:, d8, t0 + c0:t0 + c0 + cn],
                                          tz[:, :cn])
    nc.compile()
    return nc


def _get_prog(key, builder):
    if key not in _PROGS:
        _PROGS[key] = builder()
    return _PROGS[key]


def kernel(**inputs):
    hs = np.ascontiguousarray(np.asarray(inputs["hidden_states"], dtype=np.float32))
    gw = np.ascontiguousarray(np.asarray(inputs["gate_w"], dtype=np.float32))
    w1 = np.asarray(inputs["w1"], dtype=np.float32)
    w3 = np.asarray(inputs["w3"], dtype=np.float32)
    w2 = np.asarray(inputs["w2"], dtype=np.float32)
    unmerge = np.asarray(inputs["unmerge"], dtype=np.float32)

    b, s, d = hs.shape
    n = b * s
    x = hs.reshape(n, d)
    xT = np.ascontiguousarray(x.T)  # [D, n]

    # ---- Launch A: router logits ----
    ncA = _get_prog("gate", _build_gate)
    in_maps = [
        {"xt": np.ascontiguousarray(xT[:, c * NA:(c + 1) * NA]), "gw": gw}
        for c in range(NCORES)
    ]
    resA = run_bass_kernel_spmd(ncA, in_maps, CORE_IDS)
    LAST_PERF["gate"] = resA
    logits = np.concatenate([resA.results[c]["lo"] for c in range(NCORES)], axis=1)
    router_logits = np.ascontiguousarray(logits.T)  # [n, E]

    # ---- Host: top-2 routing / dispatch (the expert-parallel all-to-all) ----
    m = router_logits.max(axis=-1, keepdims=True)
    ex = np.exp(router_logits - m)
    rw = ex / ex.sum(axis=-1, keepdims=True)
    sel = np.argpartition(-rw, 1, axis=-1)[:, :2]           # top-2 experts per token
    tw = np.take_along_axis(rw, sel, axis=-1)
    tw = tw / tw.sum(axis=-1, keepdims=True)

    idx_e, wt_e = [], []
    for e in range(E):
        hit = (sel == e)
        rows = np.nonzero(hit.any(axis=-1))[0]
        wsel = np.where(hit[rows, 0], tw[rows, 0], tw[rows, 1])
        idx_e.append(rows)
        wt_e.append(wsel.astype(np.float32))
    counts = np.array([len(r) for r in idx_e])
    C = int(-(-counts.max() // P) * P)  # round up to multiple of 128

    # ---- Launch B: expert FFN chains ----
    mode = DT_MODE
    npdt = _np_dt(mode)
    ncB = _get_prog(("expert", C, mode), lambda: _build_expert(C, mode))
    in_maps = []
    for e in range(E):
        g, gi = e // GS, e % GS
        unm_e = unmerge[g][:, gi * D:(gi + 1) * D]
        w2u = w2[e] @ unm_e                                  # fold unmerge into w2
        xg = np.zeros((D, C), dtype=np.float32)
        xg[:, :counts[e]] = xT[:, idx_e[e]]
        wc = np.zeros((C,), dtype=np.float32)
        wc[:counts[e]] = wt_e[e]
        wcb = np.ascontiguousarray(np.broadcast_to(wc[None, :], (P, C)))
        w1s = np.ascontiguousarray(
            w1[e].reshape(KT, P, FT, P).transpose(2, 1, 0, 3)).astype(npdt, copy=False)
        w3s = np.ascontiguousarray(
            w3[e].reshape(KT, P, FT, P).transpose(2, 1, 0, 3)).astype(npdt, copy=False)
        w2s = np.ascontiguousarray(
            w2u.reshape(FT, P, DT8, P).transpose(2, 1, 0, 3)).astype(npdt, copy=False)
        in_maps.append({
            "xt": xg.astype(npdt, copy=False),
            "w1s": w1s, "w3s": w3s, "w2s": w2s, "wcb": wcb,
        })
    resB = run_bass_kernel_spmd(ncB, in_maps, CORE_IDS)
    LAST_PERF["expert"] = resB

    # ---- Host: unshard (scatter-add the two expert contributions per token) ----
    outT = np.zeros((d, n), dtype=np.float32)
    for e in range(E):
        outT[:, idx_e[e]] += resB.results[e]["yt"][:, :counts[e]]
    out = np.ascontiguousarray(outT.T)

    return out.reshape(b, s, d), router_logits
